# revision 6
# baseline (speedup 1.0000x reference)
"""Trainium2 Bass kernel for BoxConv2d (nn_BoxConv2d_47880295416171).

BoxConv2d is linear and separable in the input image: folding the
integral-image cumsum into the bilinear corner-interpolation gives, per
(channel c, filter f),

    out[b,c,f] = U[c,f] @ input[b,c] @ V[c,f]^T

where U[c,f,x,i] = clip(xs_b(x)-i,0,1) - clip(xs_t(x)-i,0,1) (a soft box-row
indicator, area-normalized) and V likewise along W.  U/V depend only on the
tiny [32,4] box parameters and are built on the host; the heavy work (two
128x128x128 matmuls per output plane, 2048 matmuls total) runs on the
TensorEngines.

v2 design (all-bf16):
  * host pre-transposes the input to [j, (b,c,i)] so the V-contraction's
    stationary operand needs no on-device transpose (saves 32 PE transposes
    + their PSUM evacuations per iteration).
  * bf16 input/weights/intermediate/output halve HBM traffic (5 MB/core
    vs 10 MB) and SBUF pressure; rel-err ~1e-3 << 2e-2 tolerance.
  * DRAM layouts chosen so every DMA moves >=4KB contiguous per partition
    (baseline moved 512B chunks); host un-permutes the output (cheap).
  * stage 2 batches all 8 samples per (c,f): moving dim 512 per matmul,
    PSUM evacuated in [128,1024] chunks alternating DVE/ACT.

Sharding: output-channel parallel over c (32 channels / 8 cores = 4 per
core).  Each core processes all 8 batches for its 4 channels.
"""
import contextlib

import numpy as np

import concourse.bass as bass
import concourse.tile as tile
from concourse import bacc, masks, mybir
from concourse.bass_utils import run_bass_kernel_spmd

B, C, F, H, W = 8, 32, 4, 128, 128
RH = RW = 1024.0
N_CORES = 8
CP = C // N_CORES  # channels per core = 4
FW = F * W         # 512
FBW = F * B * W    # 4096


def _build_uv(x_min, x_max, y_min, y_max):
    xmin = np.asarray(x_min, np.float64) * RH
    xmax = np.asarray(x_max, np.float64) * RH
    ymin = np.asarray(y_min, np.float64) * RW
    ymax = np.asarray(y_max, np.float64) * RW

    hx = np.arange(H, dtype=np.float64)
    wy = np.arange(W, dtype=np.float64)
    xs_t = np.clip(hx[None, None, :] + xmin[:, :, None], 0.0, H)
    xs_b = np.clip(hx[None, None, :] + xmax[:, :, None] + 1.0, 0.0, H)
    ys_l = np.clip(wy[None, None, :] + ymin[:, :, None], 0.0, W)
    ys_r = np.clip(wy[None, None, :] + ymax[:, :, None] + 1.0, 0.0, W)

    i = np.arange(H, dtype=np.float64)
    j = np.arange(W, dtype=np.float64)
    U = (np.clip(xs_b[..., None] - i, 0.0, 1.0)
         - np.clip(xs_t[..., None] - i, 0.0, 1.0))      # [C,F,H(x),H(i)]
    V = (np.clip(ys_r[..., None] - j, 0.0, 1.0)
         - np.clip(ys_l[..., None] - j, 0.0, 1.0))      # [C,F,W(y),W(j)]
    area = (xmax - xmin + 1.0) * (ymax - ymin + 1.0)
    U = U / area[:, :, None, None]
    return U.astype(np.float32), V.astype(np.float32)


def _build_bass(loop_n=1, unroll=False):
    """bf16 v2 kernel.  loop_n>1 repeats the body on-device (bench mode);
    unroll=True replicates the body inline instead of a hardware loop (for
    cost-model simulation, which cannot resolve register branches)."""
    bf = mybir.dt.bfloat16
    f32 = mybir.dt.float32
    nc = bacc.Bacc("TRN2", target_bir_lowering=False, debug=False,
                   enable_asserts=False, num_devices=N_CORES)
    it_d = nc.dram_tensor("it", [W, B * CP * H], bf, kind="ExternalInput")
    ut_d = nc.dram_tensor("ut", [H, CP * F * H], bf, kind="ExternalInput")
    vt_d = nc.dram_tensor("vt", [W, CP * FW], bf, kind="ExternalInput")
    out_d = nc.dram_tensor("out", [CP, H, FBW], bf, kind="ExternalOutput")

    with tile.TileContext(nc) as tc:
        with (
            tc.tile_pool(name="wts", bufs=1) as wpool,
            tc.tile_pool(name="inio", bufs=2) as iopool,
            tc.tile_pool(name="amat", bufs=6) as apool,
            tc.tile_pool(name="omat", bufs=6) as opool,
            tc.tile_pool(name="psa", bufs=2, space="PSUM") as psap,
            tc.tile_pool(name="pso", bufs=2, space="PSUM") as psop,
        ):
            ut_t = wpool.tile([128, CP * F * H], bf)
            nc.sync.dma_start(ut_t[:], ut_d[:])
            vt_t = wpool.tile([128, CP * FW], bf)
            nc.sync.dma_start(vt_t[:], vt_d[:])

            if unroll:
                for _ in range(loop_n):
                    _body(nc, tc, it_d, out_d, ut_t, vt_t,
                          iopool, apool, opool, psap, psop)
            else:
                rep = (tc.For_i(0, loop_n, 1, name="rep",
                                hint_engines=(mybir.EngineType.PE, mybir.EngineType.DVE,
                                              mybir.EngineType.Activation, mybir.EngineType.SP,
                                              mybir.EngineType.Pool))
                       if loop_n > 1 else contextlib.nullcontext())
                with rep:
                    _body(nc, tc, it_d, out_d, ut_t, vt_t,
                          iopool, apool, opool, psap, psop)
    nc.compile()
    return nc


def _body(nc, tc, it_d, out_d, ut_t, vt_t, iopool, apool, opool, psap, psop):
    bf = mybir.dt.bfloat16
    f32 = mybir.dt.float32

    # input [j, (b,c,i)]: two 512 KB loads on the sync ring (out-stores go
    # through gpsimd/SWDGE so the next iteration's prefetch is never queued
    # behind this iteration's stores)
    it_t = iopool.tile([128, B * CP * H], bf)
    half = B * CP * H // 2
    nc.sync.dma_start(it_t[:, :half], it_d[:, :half])
    nc.sync.dma_start(it_t[:, half:], it_d[:, half:])

    # evacuation engine split: ACT slightly faster per copy -> 17/32 share
    ev = [0]

    def evac(dst, src):
        k = ev[0]
        ev[0] += 1
        use_act = (k * 17) // 32 != ((k + 1) * 17) // 32
        if use_act:
            nc.scalar.copy(dst, src)
        else:
            nc.vector.tensor_copy(dst, src)

    def mm1_block(c, a_c, bp):
        # A[c][i, (b,f,y)] = sum_j in[b,c,i,j] * V[c,f,y,j]  (batch pair bp)
        psa = psap.tile([128, 2, FW], f32, tag="psa")
        for s in range(2):
            b = 2 * bp + s
            nc.tensor.matmul(
                psa[:, s], it_t[:, (b * CP + c) * H:(b * CP + c + 1) * H],
                vt_t[:, c * FW:(c + 1) * FW],
                start=True, stop=True,
            )
        evac(a_c[:, 2 * bp * FW:(2 * bp + 2) * FW]
             .rearrange("p (s q) -> p s q", s=2), psa[:])

    def mm2_block(c, a_c, o_c, f):
        # out[c][x, (f,b,y)] = sum_i U[c,f,x,i] * A[c][i,(b,f,y)]
        a_v = a_c[:].rearrange("p (b f y) -> p b f y", b=B, f=F)
        pso = psop.tile([128, B, W], f32, tag="pso")
        for bh in range(2):
            k = (c * F + f) * H
            nc.tensor.matmul(
                pso[:, bh * 4:(bh + 1) * 4],
                ut_t[:, k:k + H],
                a_v[:, bh * 4:(bh + 1) * 4, f, :],
                start=True, stop=True,
            )
        evac(o_c[:, f * B * W:(f + 1) * B * W]
             .rearrange("p (b y) -> p b y", b=B), pso[:])
        if f % 2 == 1:
            fh = f // 2
            nc.gpsimd.dma_start(
                out_d[c, :, fh * FBW // 2:(fh + 1) * FBW // 2],
                o_c[:, fh * FBW // 2:(fh + 1) * FBW // 2],
            )

    # software pipeline: slot c runs stage-1 of channel c interleaved with
    # stage-2 of channel c-1 so the PE always has ready work while PSUM
    # evacuations drain.
    tiles = {}
    for c in range(CP):
        a_c = apool.tile([128, B * FW], bf, tag="a", name=f"a{c}")
        o_c = opool.tile([128, FBW], bf, tag="o", name=f"o{c}")
        tiles[c] = (a_c, o_c)
    for slot in range(CP + 1):
        for step in range(4):
            if slot < CP:
                mm1_block(slot, tiles[slot][0], step)
            if slot > 0:
                c = slot - 1
                mm2_block(c, tiles[c][0], tiles[c][1], step)


def _in_maps(inputs):
    import ml_dtypes
    bf = ml_dtypes.bfloat16
    inp = np.asarray(inputs["input"], np.float32)
    U, V = _build_uv(inputs["x_min"], inputs["x_max"],
                     inputs["y_min"], inputs["y_max"])
    maps = []
    for k in range(N_CORES):
        cs = slice(CP * k, CP * (k + 1))
        # it[j, (b,c,i)] = input[b,c,i,j]
        it = np.ascontiguousarray(
            inp[:, cs].transpose(3, 0, 1, 2).reshape(W, B * CP * H)).astype(bf)
        # ut[i, (c,f,x)] = U[c,f,x,i];  vt[j, (c,f,y)] = V[c,f,y,j]
        ut = np.ascontiguousarray(
            U[cs].transpose(3, 0, 1, 2).reshape(H, CP * F * H)).astype(bf)
        vt = np.ascontiguousarray(
            V[cs].transpose(3, 0, 1, 2).reshape(W, CP * FW)).astype(bf)
        maps.append({"it": it, "ut": ut, "vt": vt})
    return maps


def _unshard(res):
    """res[k]["out"] is [CP, H, F*B*W] bf16 -> full [B, C*F, H, W] f32."""
    parts = []
    for k in range(N_CORES):
        arr = np.asarray(res.results[k]["out"]).astype(np.float32)
        arr = arr.reshape(CP, H, F, B, W).transpose(3, 0, 2, 1, 4)
        parts.append(arr.reshape(B, CP * F, H, W))
    return np.concatenate(parts, axis=1)


def run(inputs, trace=False, **kw):
    """Shard, run on 8 cores, gather. Returns (output, BassKernelResults)."""
    nc = _build_bass()
    res = run_bass_kernel_spmd(nc, _in_maps(inputs),
                               core_ids=list(range(N_CORES)),
                               trace=trace, **kw)
    return _unshard(res), res


def _null_bass():
    """Minimal 8-core program: one 64KB DMA through SBUF per core."""
    f32 = mybir.dt.float32
    nc = bacc.Bacc("TRN2", target_bir_lowering=False, debug=False,
                   enable_asserts=False, num_devices=N_CORES)
    x = nc.dram_tensor("x", [128, 128], f32, kind="ExternalInput")
    y = nc.dram_tensor("y", [128, 128], f32, kind="ExternalOutput")
    with tile.TileContext(nc) as tc:
        with tc.tile_pool(name="p", bufs=1) as p:
            t = p.tile([128, 128], f32)
            nc.sync.dma_start(t[:], x[:])
            nc.sync.dma_start(y[:], t[:])
    nc.compile()
    return nc


def _make_timed(nc, in_maps):
    """Replicate bass2jax.run_bass_via_pjrt's lowering without donation;
    return (fn, device_args) for repeated timed execution."""
    import jax
    from jax.sharding import Mesh, NamedSharding, PartitionSpec
    from jax.experimental.shard_map import shard_map
    from concourse import bass2jax, mybir as mb

    bass2jax.install_neuronx_cc_hook()
    partition_name = (nc.partition_id_tensor.name
                      if nc.partition_id_tensor else None)
    in_names, out_names, out_avals, zero_outs = [], [], [], []
    for alloc in nc.m.functions[0].allocations:
        if not isinstance(alloc, mb.MemoryLocationSet):
            continue
        name = alloc.memorylocations[0].name
        if alloc.kind == "ExternalInput":
            if name != partition_name:
                in_names.append(name)
        elif alloc.kind == "ExternalOutput":
            out_names.append(name)
            shape = tuple(alloc.tensor_shape)
            dtype = mb.dt.np(alloc.dtype)
            out_avals.append(jax.core.ShapedArray(shape, dtype))
            zero_outs.append(np.zeros(shape, dtype))
    n_params = len(in_names)
    all_names = in_names + out_names
    if partition_name is not None:
        all_names = all_names + [partition_name]

    def _body(*args):
        operands = list(args)
        if partition_name is not None:
            operands.append(bass2jax.partition_id_tensor())
        outs = bass2jax._bass_exec_p.bind(
            *operands,
            out_avals=tuple(out_avals),
            in_names=tuple(all_names),
            out_names=tuple(out_names),
            lowering_input_output_aliases=(),
            sim_require_finite=True,
            sim_require_nnan=True,
            nc=nc,
        )
        return tuple(outs)

    devices = jax.devices()[:N_CORES]
    mesh = Mesh(np.asarray(devices), ("core",))
    spec = PartitionSpec("core")
    n_all = n_params + len(out_names)
    fn = jax.jit(
        shard_map(_body, mesh=mesh, in_specs=(spec,) * n_all,
                  out_specs=(spec,) * len(out_names), check_rep=False),
        keep_unused=True,
    )
    concat_in = [
        np.concatenate([np.asarray(m[name]) for m in in_maps], axis=0)
        for name in in_names
    ]
    concat_zeros = [
        np.zeros((N_CORES * z.shape[0], *z.shape[1:]), z.dtype)
        for z in zero_outs
    ]
    sh = NamedSharding(mesh, spec)
    dev_args = [jax.device_put(a, sh) for a in concat_in + concat_zeros]
    return fn, dev_args


def bench(inputs, iters=50):
    """Time the kernel with device-resident args; subtract a null-kernel
    baseline to remove axon dispatch overhead. Returns dict of stats."""
    import time
    import jax

    stats = {}
    for tag, nc, maps in (
        ("null", _null_bass(),
         [{"x": np.zeros((128, 128), np.float32)} for _ in range(N_CORES)]),
        ("kernel", _build_bass(), _in_maps(inputs)),
    ):
        fn, args = _make_timed(nc, maps)
        jax.block_until_ready(fn(*args))  # compile + warm
        jax.block_until_ready(fn(*args))
        times = []
        for _ in range(iters):
            t0 = time.perf_counter()
            jax.block_until_ready(fn(*args))
            times.append(time.perf_counter() - t0)
        times = np.array(times)
        stats[tag] = {"mean": times.mean(), "min": times.min(),
                      "p50": float(np.median(times))}
    for k in ("mean", "min", "p50"):
        stats[f"delta_{k}_ns"] = (stats["kernel"][k] - stats["null"][k]) * 1e9
    return stats


def kernel(input, x_min, x_max, y_min, y_max):
    out, _ = run({"input": input, "x_min": x_min, "x_max": x_max,
                  "y_min": y_min, "y_max": y_max})
    return out


def bench_loop(inputs, n1=256, n2=1024, iters=30):
    """HW timing via on-device repetition: two compiles of the same program
    with loop_n=n1 and loop_n=n2; per-iteration time = delta/(n2-n1).
    Dispatch/transfer overhead cancels exactly."""
    import time
    import jax

    maps = _in_maps(inputs)
    res = {}
    for n in (n1, n2):
        nc = _build_bass(loop_n=n)
        fn, args = _make_timed(nc, maps)
        jax.block_until_ready(fn(*args))
        jax.block_until_ready(fn(*args))
        ts = []
        for _ in range(iters):
            t0 = time.perf_counter()
            jax.block_until_ready(fn(*args))
            ts.append(time.perf_counter() - t0)
        ts = np.array(ts)
        res[n] = {"p50": float(np.median(ts)), "mean": ts.mean(),
                  "min": ts.min()}
        print(f"  loop_n={n}: p50 {res[n]['p50']*1e3:.1f}ms "
              f"min {res[n]['min']*1e3:.1f}ms mean {res[n]['mean']*1e3:.1f}ms")
    dn = n2 - n1
    return {k: (res[n2][k] - res[n1][k]) / dn * 1e9 for k in ("p50", "mean", "min")}


# revision 9
# speedup vs baseline: 1.6276x; 1.6276x over previous
"""Trainium2 Bass kernel for BoxConv2d (nn_BoxConv2d_47880295416171).

BoxConv2d is linear and separable in the input image: folding the
integral-image cumsum into the bilinear corner-interpolation gives, per
(channel c, filter f),

    out[b,c,f] = U[c,f] @ input[b,c] @ V[c,f]^T

where U[c,f,x,i] = clip(xs_b(x)-i,0,1) - clip(xs_t(x)-i,0,1) (a soft box-row
indicator, area-normalized) and V likewise along W.  U/V depend only on the
tiny [32,4] box parameters and are built on the host; the heavy work (two
128x128x128 matmuls per output plane, 2048 matmuls total) runs on the
TensorEngines.

v2 design (all-bf16):
  * host pre-transposes the input to [j, (b,c,i)] so the V-contraction's
    stationary operand needs no on-device transpose (saves 32 PE transposes
    + their PSUM evacuations per iteration).
  * bf16 input/weights/intermediate/output halve HBM traffic (5 MB/core
    vs 10 MB) and SBUF pressure; rel-err ~1e-3 << 2e-2 tolerance.
  * DRAM layouts chosen so every DMA moves >=4KB contiguous per partition
    (baseline moved 512B chunks); host un-permutes the output (cheap).
  * stage 2 batches all 8 samples per (c,f): moving dim 512 per matmul,
    PSUM evacuated in [128,1024] chunks alternating DVE/ACT.

Sharding: output-channel parallel over c (32 channels / 8 cores = 4 per
core).  Each core processes all 8 batches for its 4 channels.
"""
import contextlib

import numpy as np

import concourse.bass as bass
import concourse.tile as tile
from concourse import bacc, masks, mybir
from concourse.bass_utils import run_bass_kernel_spmd

B, C, F, H, W = 8, 32, 4, 128, 128
RH = RW = 1024.0
N_CORES = 8
CP = C // N_CORES  # channels per core = 4
FW = F * W         # 512
FBW = F * B * W    # 4096


def _build_uv(x_min, x_max, y_min, y_max):
    xmin = np.asarray(x_min, np.float64) * RH
    xmax = np.asarray(x_max, np.float64) * RH
    ymin = np.asarray(y_min, np.float64) * RW
    ymax = np.asarray(y_max, np.float64) * RW

    hx = np.arange(H, dtype=np.float64)
    wy = np.arange(W, dtype=np.float64)
    xs_t = np.clip(hx[None, None, :] + xmin[:, :, None], 0.0, H)
    xs_b = np.clip(hx[None, None, :] + xmax[:, :, None] + 1.0, 0.0, H)
    ys_l = np.clip(wy[None, None, :] + ymin[:, :, None], 0.0, W)
    ys_r = np.clip(wy[None, None, :] + ymax[:, :, None] + 1.0, 0.0, W)

    i = np.arange(H, dtype=np.float64)
    j = np.arange(W, dtype=np.float64)
    U = (np.clip(xs_b[..., None] - i, 0.0, 1.0)
         - np.clip(xs_t[..., None] - i, 0.0, 1.0))      # [C,F,H(x),H(i)]
    V = (np.clip(ys_r[..., None] - j, 0.0, 1.0)
         - np.clip(ys_l[..., None] - j, 0.0, 1.0))      # [C,F,W(y),W(j)]
    area = (xmax - xmin + 1.0) * (ymax - ymin + 1.0)
    U = U / area[:, :, None, None]
    return U.astype(np.float32), V.astype(np.float32)


def _build_bass(loop_n=1, unroll=False, unroll_inner=8):
    """bf16 v2 kernel.  loop_n>1 repeats the body on-device (bench mode);
    unroll=True replicates the body inline instead of a hardware loop (for
    cost-model simulation, which cannot resolve register branches).
    unroll_inner: bodies per hardware-loop iteration — the For_i back-edge
    is a full cross-engine barrier (semaphore state must recycle), so
    consecutive bodies only pipeline when unrolled inside one iteration."""
    bf = mybir.dt.bfloat16
    f32 = mybir.dt.float32
    nc = bacc.Bacc("TRN2", target_bir_lowering=False, debug=False,
                   enable_asserts=False, num_devices=N_CORES)
    it_d = nc.dram_tensor("it", [W, B * CP * H], bf, kind="ExternalInput")
    ut_d = nc.dram_tensor("ut", [H, CP * F * H], bf, kind="ExternalInput")
    vt_d = nc.dram_tensor("vt", [W, CP * FW], bf, kind="ExternalInput")
    out_d = nc.dram_tensor("out", [CP, H, FBW], bf, kind="ExternalOutput")

    with tile.TileContext(nc) as tc:
        with (
            tc.tile_pool(name="wts", bufs=1) as wpool,
            tc.tile_pool(name="inio", bufs=3) as iopool,
            tc.tile_pool(name="amat", bufs=8) as apool,
            tc.tile_pool(name="omat", bufs=8) as opool,
            tc.tile_pool(name="psa", bufs=2, space="PSUM") as psap,
            tc.tile_pool(name="pso", bufs=2, space="PSUM") as psop,
        ):
            ut_t = wpool.tile([128, CP * F * H], bf)
            nc.sync.dma_start(ut_t[:], ut_d[:])
            vt_t = wpool.tile([128, CP * FW], bf)
            nc.sync.dma_start(vt_t[:], vt_d[:])

            if unroll:
                for _ in range(loop_n):
                    _body(nc, tc, it_d, out_d, ut_t, vt_t,
                          iopool, apool, opool, psap, psop)
            elif loop_n > 1:
                inner = unroll_inner if loop_n % unroll_inner == 0 else 1
                with tc.For_i(0, loop_n // inner, 1, name="rep",
                              hint_engines=(mybir.EngineType.PE, mybir.EngineType.DVE,
                                            mybir.EngineType.Activation, mybir.EngineType.SP,
                                            mybir.EngineType.Pool)):
                    for _ in range(inner):
                        _body(nc, tc, it_d, out_d, ut_t, vt_t,
                              iopool, apool, opool, psap, psop)
            else:
                _body(nc, tc, it_d, out_d, ut_t, vt_t,
                      iopool, apool, opool, psap, psop)
    nc.compile()
    return nc


def _body(nc, tc, it_d, out_d, ut_t, vt_t, iopool, apool, opool, psap, psop):
    bf = mybir.dt.bfloat16
    f32 = mybir.dt.float32

    # input [j, (b,c,i)]: two 512 KB loads on the sync ring (out-stores go
    # through gpsimd/SWDGE so the next iteration's prefetch is never queued
    # behind this iteration's stores)
    it_t = iopool.tile([128, B * CP * H], bf)
    half = B * CP * H // 2
    nc.sync.dma_start(it_t[:, :half], it_d[:, :half])
    nc.sync.dma_start(it_t[:, half:], it_d[:, half:])

    # evacuation engine split: ACT slightly faster per copy -> 17/32 share
    ev = [0]

    def evac(dst, src):
        k = ev[0]
        ev[0] += 1
        use_act = (k * 17) // 32 != ((k + 1) * 17) // 32
        if use_act:
            nc.scalar.copy(dst, src)
        else:
            nc.vector.tensor_copy(dst, src)

    def mm1_block(c, a_c, bp):
        # A[c][i, (b,f,y)] = sum_j in[b,c,i,j] * V[c,f,y,j]  (batch pair bp)
        psa = psap.tile([128, 2, FW], f32, tag="psa")
        for s in range(2):
            b = 2 * bp + s
            nc.tensor.matmul(
                psa[:, s], it_t[:, (b * CP + c) * H:(b * CP + c + 1) * H],
                vt_t[:, c * FW:(c + 1) * FW],
                start=True, stop=True,
            )
        evac(a_c[:, 2 * bp * FW:(2 * bp + 2) * FW]
             .rearrange("p (s q) -> p s q", s=2), psa[:])

    def mm2_block(c, a_c, o_c, f):
        # out[c][x, (f,b,y)] = sum_i U[c,f,x,i] * A[c][i,(b,f,y)]
        a_v = a_c[:].rearrange("p (b f y) -> p b f y", b=B, f=F)
        pso = psop.tile([128, B, W], f32, tag="pso")
        for bh in range(2):
            k = (c * F + f) * H
            nc.tensor.matmul(
                pso[:, bh * 4:(bh + 1) * 4],
                ut_t[:, k:k + H],
                a_v[:, bh * 4:(bh + 1) * 4, f, :],
                start=True, stop=True,
            )
        evac(o_c[:, f * B * W:(f + 1) * B * W]
             .rearrange("p (b y) -> p b y", b=B), pso[:])
        if f % 2 == 1:
            fh = f // 2
            nc.gpsimd.dma_start(
                out_d[c, :, fh * FBW // 2:(fh + 1) * FBW // 2],
                o_c[:, fh * FBW // 2:(fh + 1) * FBW // 2],
            )

    # software pipeline: slot c runs stage-1 of channel c interleaved with
    # stage-2 of channel c-1 so the PE always has ready work while PSUM
    # evacuations drain.
    tiles = {}
    for c in range(CP):
        a_c = apool.tile([128, B * FW], bf, tag="a", name=f"a{c}")
        o_c = opool.tile([128, FBW], bf, tag="o", name=f"o{c}")
        tiles[c] = (a_c, o_c)
    for slot in range(CP + 1):
        for step in range(4):
            if slot < CP:
                mm1_block(slot, tiles[slot][0], step)
            if slot > 0:
                c = slot - 1
                mm2_block(c, tiles[c][0], tiles[c][1], step)


def _in_maps(inputs):
    import ml_dtypes
    bf = ml_dtypes.bfloat16
    inp = np.asarray(inputs["input"], np.float32)
    U, V = _build_uv(inputs["x_min"], inputs["x_max"],
                     inputs["y_min"], inputs["y_max"])
    maps = []
    for k in range(N_CORES):
        cs = slice(CP * k, CP * (k + 1))
        # it[j, (b,c,i)] = input[b,c,i,j]
        it = np.ascontiguousarray(
            inp[:, cs].transpose(3, 0, 1, 2).reshape(W, B * CP * H)).astype(bf)
        # ut[i, (c,f,x)] = U[c,f,x,i];  vt[j, (c,f,y)] = V[c,f,y,j]
        ut = np.ascontiguousarray(
            U[cs].transpose(3, 0, 1, 2).reshape(H, CP * F * H)).astype(bf)
        vt = np.ascontiguousarray(
            V[cs].transpose(3, 0, 1, 2).reshape(W, CP * FW)).astype(bf)
        maps.append({"it": it, "ut": ut, "vt": vt})
    return maps


def _unshard(res):
    """res[k]["out"] is [CP, H, F*B*W] bf16 -> full [B, C*F, H, W] f32."""
    parts = []
    for k in range(N_CORES):
        arr = np.asarray(res.results[k]["out"]).astype(np.float32)
        arr = arr.reshape(CP, H, F, B, W).transpose(3, 0, 2, 1, 4)
        parts.append(arr.reshape(B, CP * F, H, W))
    return np.concatenate(parts, axis=1)


def run(inputs, trace=False, **kw):
    """Shard, run on 8 cores, gather. Returns (output, BassKernelResults)."""
    nc = _build_bass()
    res = run_bass_kernel_spmd(nc, _in_maps(inputs),
                               core_ids=list(range(N_CORES)),
                               trace=trace, **kw)
    return _unshard(res), res


def _null_bass():
    """Minimal 8-core program: one 64KB DMA through SBUF per core."""
    f32 = mybir.dt.float32
    nc = bacc.Bacc("TRN2", target_bir_lowering=False, debug=False,
                   enable_asserts=False, num_devices=N_CORES)
    x = nc.dram_tensor("x", [128, 128], f32, kind="ExternalInput")
    y = nc.dram_tensor("y", [128, 128], f32, kind="ExternalOutput")
    with tile.TileContext(nc) as tc:
        with tc.tile_pool(name="p", bufs=1) as p:
            t = p.tile([128, 128], f32)
            nc.sync.dma_start(t[:], x[:])
            nc.sync.dma_start(y[:], t[:])
    nc.compile()
    return nc


def _make_timed(nc, in_maps):
    """Replicate bass2jax.run_bass_via_pjrt's lowering without donation;
    return (fn, device_args) for repeated timed execution."""
    import jax
    from jax.sharding import Mesh, NamedSharding, PartitionSpec
    from jax.experimental.shard_map import shard_map
    from concourse import bass2jax, mybir as mb

    bass2jax.install_neuronx_cc_hook()
    partition_name = (nc.partition_id_tensor.name
                      if nc.partition_id_tensor else None)
    in_names, out_names, out_avals, zero_outs = [], [], [], []
    for alloc in nc.m.functions[0].allocations:
        if not isinstance(alloc, mb.MemoryLocationSet):
            continue
        name = alloc.memorylocations[0].name
        if alloc.kind == "ExternalInput":
            if name != partition_name:
                in_names.append(name)
        elif alloc.kind == "ExternalOutput":
            out_names.append(name)
            shape = tuple(alloc.tensor_shape)
            dtype = mb.dt.np(alloc.dtype)
            out_avals.append(jax.core.ShapedArray(shape, dtype))
            zero_outs.append(np.zeros(shape, dtype))
    n_params = len(in_names)
    all_names = in_names + out_names
    if partition_name is not None:
        all_names = all_names + [partition_name]

    def _body(*args):
        operands = list(args)
        if partition_name is not None:
            operands.append(bass2jax.partition_id_tensor())
        outs = bass2jax._bass_exec_p.bind(
            *operands,
            out_avals=tuple(out_avals),
            in_names=tuple(all_names),
            out_names=tuple(out_names),
            lowering_input_output_aliases=(),
            sim_require_finite=True,
            sim_require_nnan=True,
            nc=nc,
        )
        return tuple(outs)

    devices = jax.devices()[:N_CORES]
    mesh = Mesh(np.asarray(devices), ("core",))
    spec = PartitionSpec("core")
    n_all = n_params + len(out_names)
    fn = jax.jit(
        shard_map(_body, mesh=mesh, in_specs=(spec,) * n_all,
                  out_specs=(spec,) * len(out_names), check_rep=False),
        keep_unused=True,
    )
    concat_in = [
        np.concatenate([np.asarray(m[name]) for m in in_maps], axis=0)
        for name in in_names
    ]
    concat_zeros = [
        np.zeros((N_CORES * z.shape[0], *z.shape[1:]), z.dtype)
        for z in zero_outs
    ]
    sh = NamedSharding(mesh, spec)
    dev_args = [jax.device_put(a, sh) for a in concat_in + concat_zeros]
    return fn, dev_args


def bench(inputs, iters=50):
    """Time the kernel with device-resident args; subtract a null-kernel
    baseline to remove axon dispatch overhead. Returns dict of stats."""
    import time
    import jax

    stats = {}
    for tag, nc, maps in (
        ("null", _null_bass(),
         [{"x": np.zeros((128, 128), np.float32)} for _ in range(N_CORES)]),
        ("kernel", _build_bass(), _in_maps(inputs)),
    ):
        fn, args = _make_timed(nc, maps)
        jax.block_until_ready(fn(*args))  # compile + warm
        jax.block_until_ready(fn(*args))
        times = []
        for _ in range(iters):
            t0 = time.perf_counter()
            jax.block_until_ready(fn(*args))
            times.append(time.perf_counter() - t0)
        times = np.array(times)
        stats[tag] = {"mean": times.mean(), "min": times.min(),
                      "p50": float(np.median(times))}
    for k in ("mean", "min", "p50"):
        stats[f"delta_{k}_ns"] = (stats["kernel"][k] - stats["null"][k]) * 1e9
    return stats


def kernel(input, x_min, x_max, y_min, y_max):
    out, _ = run({"input": input, "x_min": x_min, "x_max": x_max,
                  "y_min": y_min, "y_max": y_max})
    return out


def bench_loop(inputs, n1=256, n2=1024, iters=30):
    """HW timing via on-device repetition: two compiles of the same program
    with loop_n=n1 and loop_n=n2; per-iteration time = delta/(n2-n1).
    Dispatch/transfer overhead cancels exactly."""
    import time
    import jax

    maps = _in_maps(inputs)
    res = {}
    for n in (n1, n2):
        nc = _build_bass(loop_n=n)
        fn, args = _make_timed(nc, maps)
        jax.block_until_ready(fn(*args))
        jax.block_until_ready(fn(*args))
        ts = []
        for _ in range(iters):
            t0 = time.perf_counter()
            jax.block_until_ready(fn(*args))
            ts.append(time.perf_counter() - t0)
        ts = np.array(ts)
        res[n] = {"p50": float(np.median(ts)), "mean": ts.mean(),
                  "min": ts.min()}
        print(f"  loop_n={n}: p50 {res[n]['p50']*1e3:.1f}ms "
              f"min {res[n]['min']*1e3:.1f}ms mean {res[n]['mean']*1e3:.1f}ms")
    dn = n2 - n1
    return {k: (res[n2][k] - res[n1][k]) / dn * 1e9 for k in ("p50", "mean", "min")}
